# revision 1
# baseline (speedup 1.0000x reference)
"""Trainium2 Bass kernel for nn_ConvPolicy (tiny per-row conv policy net).

Network (per row of x[B, 18], all fp32):
  obs = x[:, :4]; j = x[:, 4:11]; jd = x[:, 11:18]
  u    = relu(obs @ Wo.T + bo)                          # [2]
  c1_t = relu(sum_k x[4+2t+k]*cw0k + x[11+2t+k]*cw1k + cb), t=0..2
  s_t  = relu(c1_t*c2w0 + c1_{t+1}*c2w1 + c2b), t=0,1
  e_t  = relu((u0+s0)*we_t0 + (u1+s1)*we_t1 + be_t), t=0,1
  d0 = relu(e0*v0 + d1b); d1 = relu(e0*v1 + e1*v0 + d1b); d2 = relu(e1*v1 + d1b)
  y0=g0*w0+b; y1=g0*w1+b; y2=g0*w2+g1*w0+b; y3=g1*w1+b;
  y4=g1*w2+g2*w0+b; y5=g2*w1+b; y6=g2*w2+b            # [7]

Strategy: pure data parallel over 8 cores. Row-major layout in SBUF:
tile [128 partitions, c rows * 18 feats], per-row dot products done as
chains of fused scalar_tensor_tensor (DVE) accumulations, first terms +
relus + single-term affine ops on ScalarE (ACT: out = func(scale*x+bias)).
Output written row-major [128, c*7] so both DMAs are fully coalesced.

Weights are baked into the instruction stream as immediates at build time.
"""

import numpy as np

B = 2_000_000
N_CORES = 8
P = 128
C = 500   # rows per partition per tile
T = 4     # tiles per core
ROWS_PER_CORE = P * C * T          # 256_000
PADDED = ROWS_PER_CORE * N_CORES   # 2_048_000


def _build(weights: dict, rows: int, c: int, n_tiles: int, reps: int = 1,
           mode: str = "full"):
    import concourse.bass as bass
    import concourse.mybir as mybir
    from concourse.tile import TileContext

    f32 = mybir.dt.float32
    MULT = mybir.AluOpType.mult
    ADD = mybir.AluOpType.add
    RELU = mybir.ActivationFunctionType.Relu
    IDENT = mybir.ActivationFunctionType.Identity

    wo = weights["fc_obs_w"]          # [2, 4]
    bo = weights["fc_obs_b"]          # [2]
    we = weights["fc_emb_w"]          # [2, 2]
    be = weights["fc_emb_b"]          # [2]
    cw = weights["conv1_w"][0]        # [2, 3]
    cb = float(weights["conv1_b"][0])
    c2 = weights["conv2_w"][0, 0]     # [2]
    c2b = float(weights["conv2_b"][0])
    dv = weights["deconv1_w"][0, 0]   # [2]
    d1b = float(weights["deconv1_b"][0])
    dw = weights["deconv2_w"][0, 0]   # [3]
    d2b = float(weights["deconv2_b"][0])

    nc = bass.Bass()
    x = nc.declare_dram_parameter("x", [rows, 18], f32, isOutput=False)
    y = nc.declare_dram_parameter("y", [rows, 7], f32, isOutput=True)
    xv = x.rearrange("(t p c) f -> t p (c f)", t=n_tiles, p=P, c=c)
    yv = y.rearrange("(t p c) g -> t p (c g)", t=n_tiles, p=P, c=c)

    def stt(out, in0, s, in1):
        # out = in0 * s + in1   (fused MAC on VectorE)
        nc.vector.scalar_tensor_tensor(
            out=out, in0=in0, scalar=float(s), in1=in1, op0=MULT, op1=ADD
        )

    def ts2(out, in0, s, b):
        # out = in0 * s + b     (VectorE, dual-op tensor_scalar, imm scalars)
        nc.vector.tensor_scalar(
            out=out, in0=in0, scalar1=float(s), scalar2=float(b),
            op0=MULT, op1=ADD,
        )

    # Distinct bias values needing [128,1] const APs (ScalarE bias operand).
    bias_vals = sorted(
        {float(v) for v in (bo[0], bo[1], c2b, be[0], be[1], d1b, d2b)}
    )
    bias_ap = {}

    def aff(out, in_, s, b):
        # out = in_ * s + b     (ScalarE)
        nc.scalar.activation(out, in_, IDENT, bias=bias_ap[float(b)], scale=float(s))

    def relu(out, in_, s=1.0, b=0.0):
        # out = relu(in_ * s + b)  (ScalarE)
        bias = bias_ap[float(b)] if b else 0.0
        nc.scalar.activation(out, in_, RELU, bias=bias, scale=float(s))

    with TileContext(nc) as tc:
        with (
            tc.tile_pool(name="const", bufs=1) as cpool,
            tc.tile_pool(name="xin", bufs=2) as xp,
            tc.tile_pool(name="yout", bufs=2) as ypool,
            tc.tile_pool(name="mid", bufs=2) as mp,
        ):
            btile = cpool.tile([P, len(bias_vals)], f32)
            scratch = cpool.tile([P, 1], f32)
            for i, v in enumerate(bias_vals):
                nc.vector.memset(btile[:, i:i + 1], v)
                bias_ap[v] = btile[:, i:i + 1]
            for t in [t_ for _ in range(reps) for t_ in range(n_tiles)]:
                xt = xp.tile([P, 18 * c], f32, tag="x")
                nc.sync.dma_start(out=xt[:], in_=xv[t])
                X = xt[:].rearrange("p (c f) -> p c f", f=18)   # [P, c, 18]
                X3 = xt[:].rearrange("p (c f) -> p f c", f=18)  # [P, 18, c]

                U = mp.tile([P, 2 * c], f32, tag="U")
                C1 = mp.tile([P, 3 * c], f32, tag="C1")
                S = mp.tile([P, 2 * c], f32, tag="S")
                E = mp.tile([P, 2 * c], f32, tag="E")
                D = mp.tile([P, 3 * c], f32, tag="D")
                TMP = mp.tile([P, 2 * c], f32, tag="TMP")
                Y = ypool.tile([P, 7 * c], f32, tag="y")

                if mode == "dma":
                    nc.sync.dma_start(out=yv[t], in_=xt[:, 0:7 * c])
                    continue

                # ScalarE wait-absorber: first ACT op of the iteration takes
                # the DMA-completion wait so later ACT ops carry <=1 wait
                # (Activation ISA slot limit).
                nc.scalar.copy(scratch[:], xt[:, 0:1])

                # --- fc_obs: u_a, u_b (pre-relu; first term on ACT) ---
                n_fc = 2 if mode == "half" else 4
                for ch in range(2):
                    dst = U[:, ch * c:(ch + 1) * c]
                    aff(dst, X[:, :, 0], wo[ch, 0], bo[ch])
                    for i in range(1, n_fc):
                        stt(dst, X[:, :, i], wo[ch, i], dst)

                # --- conv1: all 3 channels per tap via strided views (DVE) ---
                C1v = C1[:].rearrange("p (t c) -> p t c", t=3)  # [P, 3, c]
                ts2(C1v, X3[:, 4:10:2, :], cw[0, 0], cb)
                stt(C1v, X3[:, 5:11:2, :], cw[0, 1], C1v)
                if mode != "half":
                    stt(C1v, X3[:, 6:12:2, :], cw[0, 2], C1v)
                    stt(C1v, X3[:, 11:17:2, :], cw[1, 0], C1v)
                    stt(C1v, X3[:, 12:18:2, :], cw[1, 1], C1v)
                    stt(C1v, X3[:, 13:18:2, :], cw[1, 2], C1v)

                relu(U[:], U[:])
                relu(C1[:], C1[:])

                # --- conv2: s_t = c1_t*c2[0] + c1_{t+1}*c2[1] + c2b ---
                aff(S[:], C1[:, 0:2 * c], c2[0], c2b)
                stt(S[:], C1[:, c:3 * c], c2[1], S[:])
                relu(S[:], S[:])

                # --- u + s ---
                nc.vector.tensor_add(out=S[:], in0=U[:], in1=S[:])

                # --- fc_emb ---
                for ch in range(2):
                    dst = E[:, ch * c:(ch + 1) * c]
                    aff(dst, S[:, 0:c], we[ch, 0], be[ch])
                    stt(dst, S[:, c:2 * c], we[ch, 1], dst)
                relu(E[:], E[:])

                # --- deconv1 -> G (=D after relu) ---
                relu(D[:, 0:c], E[:, 0:c], s=dv[0], b=d1b)              # d0 fused
                relu(D[:, 2 * c:3 * c], E[:, c:2 * c], s=dv[1], b=d1b)  # d2 fused
                ts2(D[:, c:2 * c], E[:, 0:c], dv[1], d1b)
                stt(D[:, c:2 * c], E[:, c:2 * c], dv[0], D[:, c:2 * c])
                relu(D[:, c:2 * c], D[:, c:2 * c])

                # --- deconv2 -> Y row-major (c, 7) ---
                # ACT writes y0/y1/y3/y5/y6 (only Y reader is the out-DMA);
                # DVE computes y2/y4 in TMP, then one strided copy into Y.
                Yv = Y[:].rearrange("p (c g) -> p g c", g=7)  # [P, 7, c]
                Gv = D[:].rearrange("p (l c) -> p l c", l=3)  # [P, 3, c]
                aff(Yv[:, 0, :], D[:, 0:c], dw[0], d2b)       # y0
                aff(Yv[:, 1:6:2, :], Gv, dw[1], d2b)          # y1, y3, y5
                aff(Yv[:, 6, :], D[:, 2 * c:3 * c], dw[2], d2b)  # y6
                ts2(TMP[:, 0:c], D[:, 0:c], dw[2], d2b)       # y2 = g0*w2 + b
                stt(TMP[:, 0:c], D[:, c:2 * c], dw[0], TMP[:, 0:c])
                ts2(TMP[:, c:2 * c], D[:, c:2 * c], dw[2], d2b)  # y4
                stt(TMP[:, c:2 * c], D[:, 2 * c:3 * c], dw[0], TMP[:, c:2 * c])
                nc.vector.tensor_copy(
                    out=Yv[:, 2:5:2, :],
                    in_=TMP[:].rearrange("p (l c) -> p l c", l=2),
                )

                nc.sync.dma_start(out=yv[t], in_=Y[:])

    _split_multi_waits(nc)
    return nc


def _split_multi_waits(nc):
    """Walrus codegen accepts at most ONE sync-wait per instruction; hoist
    extra waits onto standalone same-engine NoOps placed just before."""
    import concourse.mybir as mybir

    n = 0
    for fn in nc.m.functions:
        for bb in fn.blocks:
            out = []
            for ins in bb.instructions:
                si = getattr(ins, "sync_info", None)
                waits = list(si.on_wait) if si and si.on_wait else []
                if len(waits) > 1:
                    for w in waits[:-1]:
                        nop = mybir.InstNoOp(name=f"waitnop-{n}", ins=[], outs=[])
                        n += 1
                        nop.engine = ins.engine
                        nop.sync_info = mybir.SyncInfo(on_wait=[w], on_update=[])
                        out.append(nop)
                    ins.sync_info = mybir.SyncInfo(
                        on_wait=[waits[-1]], on_update=list(si.on_update or [])
                    )
                out.append(ins)
            bb.instructions = out


LAST_RESULTS = None  # test harness introspection (exec_time_ns, profile)


def _run(nc, in_maps, core_ids, trace=False):
    global LAST_RESULTS
    from concourse.bass_utils import run_bass_kernel_spmd

    LAST_RESULTS = run_bass_kernel_spmd(nc, in_maps, core_ids, trace=trace)
    return LAST_RESULTS


def kernel(**inputs) -> np.ndarray:
    x = np.asarray(inputs["x"], dtype=np.float32)
    weights = {
        k: np.asarray(v, dtype=np.float32) for k, v in inputs.items() if k != "x"
    }
    assert x.shape == (B, 18), x.shape

    nc = _build(weights, ROWS_PER_CORE, C, T)

    xp = np.zeros((PADDED, 18), dtype=np.float32)
    xp[:B] = x
    shards = xp.reshape(N_CORES, ROWS_PER_CORE, 18)
    in_maps = [{"x": np.ascontiguousarray(shards[i])} for i in range(N_CORES)]

    res = _run(nc, in_maps, list(range(N_CORES)))
    outs = [np.asarray(res.results[i]["y"]) for i in range(N_CORES)]
    y = np.concatenate(outs, axis=0)[:B]
    return np.ascontiguousarray(y.reshape(B, 1, 7))



# revision 5
# speedup vs baseline: 1.0209x; 1.0209x over previous
"""Trainium2 Bass kernel for nn_ConvPolicy (tiny per-row conv policy net).

Network (per row of x[B, 18], all fp32):
  obs = x[:, :4]; j = x[:, 4:11]; jd = x[:, 11:18]
  u    = relu(obs @ Wo.T + bo)                          # [2]
  c1_t = relu(sum_k x[4+2t+k]*cw0k + x[11+2t+k]*cw1k + cb), t=0..2
  s_t  = relu(c1_t*c2w0 + c1_{t+1}*c2w1 + c2b), t=0,1
  e_t  = relu((u0+s0)*we_t0 + (u1+s1)*we_t1 + be_t), t=0,1
  d0 = relu(e0*v0 + d1b); d1 = relu(e0*v1 + e1*v0 + d1b); d2 = relu(e1*v1 + d1b)
  y0=g0*w0+b; y1=g0*w1+b; y2=g0*w2+g1*w0+b; y3=g1*w1+b;
  y4=g1*w2+g2*w0+b; y5=g2*w1+b; y6=g2*w2+b            # [7]

Strategy: pure data parallel over 8 cores; row-major SBUF tiles
[128, c*18] so both DMAs are fully coalesced.  All access patterns are
2D ([P, c] slices, stride-18 reads / stride-7 writes) — 3D strided APs
run ~2x slower per element on DVE/ACT.  GPSIMD elementwise is ~14
ns/elem on this toolchain (no partition vectorization), and custom DVE
ops fail walrus codegen ("ISA wrong length"), so compute is split
between VectorE (all MAC accumulation chains via scalar_tensor_tensor,
plus cheap 2x-mode tensor_scalar relus) and ScalarE (single-input
affine first-taps, d0/d2, and 7 y-writes).  relu(S)+relu(U) is fused
into one DVE STT via op0=max / op1=add.  U and C1 share one buffer so
their relu is a single 5c-wide op.  Tile sizes ramp up/down so the
first tile's input DMA and the last tile's output DMA barely stall the
pipeline.  Weights are baked in as immediates at build time.
"""

import numpy as np

B = 2_000_000
N_CORES = 8
P = 128
C_LIST = (192, 448, 672, 642)      # rows/partition per tile; sum = 1954
ROWS_PER_CORE = P * sum(C_LIST)    # 250_112
PADDED = ROWS_PER_CORE * N_CORES   # 2_000_896


def _build(weights: dict, c_list=C_LIST):
    import concourse.bass as bass
    import concourse.mybir as mybir
    from concourse.tile import TileContext

    f32 = mybir.dt.float32
    MULT = mybir.AluOpType.mult
    ADD = mybir.AluOpType.add
    MAX = mybir.AluOpType.max
    RELU = mybir.ActivationFunctionType.Relu
    IDENT = mybir.ActivationFunctionType.Identity

    wo = weights["fc_obs_w"]          # [2, 4]
    bo = weights["fc_obs_b"]          # [2]
    we = weights["fc_emb_w"]          # [2, 2]
    be = weights["fc_emb_b"]          # [2]
    cw = weights["conv1_w"][0]        # [2, 3]
    cb = float(weights["conv1_b"][0])
    c2 = weights["conv2_w"][0, 0]     # [2]
    c2b = float(weights["conv2_b"][0])
    dv = weights["deconv1_w"][0, 0]   # [2]
    d1b = float(weights["deconv1_b"][0])
    dw = weights["deconv2_w"][0, 0]   # [3]
    d2b = float(weights["deconv2_b"][0])

    rows = P * sum(c_list)
    nc = bass.Bass()
    x = nc.declare_dram_parameter("x", [rows, 18], f32, isOutput=False)
    y = nc.declare_dram_parameter("y", [rows, 7], f32, isOutput=True)

    def vstt(out, in0, s, in1, op0=MULT, op1=ADD):
        # out = (in0 op0 s) op1 in1   (VectorE fused MAC)
        nc.vector.scalar_tensor_tensor(
            out=out, in0=in0, scalar=float(s), in1=in1, op0=op0, op1=op1)

    def vrelu(ap):
        # in-place relu on VectorE (tensor_scalar 2x mode, fp32)
        nc.vector.tensor_scalar(
            out=ap, in0=ap, scalar1=1.0, scalar2=0.0, op0=MULT, op1=MAX)

    bias_vals = sorted({float(v) for v in
                        (0.0, bo[0], bo[1], cb, c2b, be[0], be[1], d1b, d2b)})
    bias_ap = {}

    with TileContext(nc) as tc:
        with (
            tc.tile_pool(name="const", bufs=1) as cpool,
            tc.tile_pool(name="xin", bufs=2) as xp,
            tc.tile_pool(name="yout", bufs=2) as ypool,
            tc.tile_pool(name="mid", bufs=2) as mp,
        ):
            btile = cpool.tile([P, len(bias_vals)], f32)
            scratch = cpool.tile([P, 1], f32)
            for i, v in enumerate(bias_vals):
                nc.vector.memset(btile[:, i:i + 1], v)
                bias_ap[v] = btile[:, i:i + 1]

            def aff(out, in_, s, b, func=IDENT):
                # out = func(in_ * s + b)   (ScalarE)
                nc.scalar.activation(out, in_, func, bias=bias_ap[float(b)],
                                     scale=float(s))

            row0 = 0
            for t, c in enumerate(c_list):
                xd = x[row0:row0 + P * c, :].rearrange(
                    "(p c) f -> p (c f)", p=P)
                yd = y[row0:row0 + P * c, :].rearrange(
                    "(p c) g -> p (c g)", p=P)
                row0 += P * c

                xt = xp.tile([P, 18 * c], f32, tag="x")
                nc.sync.dma_start(out=xt[:], in_=xd)
                X = xt[:].rearrange("p (c f) -> p c f", f=18)   # [P, c, 18]

                # U and C1 share one tile so their relu is one 5c-wide op
                UC = mp.tile([P, 5 * c], f32, tag="UC")
                U = UC[:, 0:2 * c]
                C1 = UC[:, 2 * c:5 * c]
                S = mp.tile([P, 2 * c], f32, tag="S")
                E = mp.tile([P, 2 * c], f32, tag="E")
                D = mp.tile([P, 3 * c], f32, tag="D")
                Y = ypool.tile([P, 7 * c], f32, tag="y")
                Yv = Y[:].rearrange("p (c g) -> p g c", g=7)    # [P, 7, c]

                # ScalarE wait-absorber: first ACT op of the iteration
                # takes the DMA-completion wait.
                nc.scalar.copy(scratch[:], xt[:, 0:1])

                # --- layer-1 first taps (ScalarE) ---
                aff(U[:, 0:c], X[:, :, 0], wo[0, 0], bo[0])
                aff(U[:, c:2 * c], X[:, :, 0], wo[1, 0], bo[1])
                for ch in range(3):
                    aff(C1[:, ch * c:(ch + 1) * c], X[:, :, 4 + 2 * ch],
                        cw[0, 0], cb)

                # --- layer-1 accumulation chains (DVE) ---
                for ch in range(2):
                    dst = U[:, ch * c:(ch + 1) * c]
                    for i in range(1, 4):
                        vstt(dst, X[:, :, i], wo[ch, i], dst)
                for ch in range(3):
                    dst = C1[:, ch * c:(ch + 1) * c]
                    vstt(dst, X[:, :, 5 + 2 * ch], cw[0, 1], dst)
                    vstt(dst, X[:, :, 6 + 2 * ch], cw[0, 2], dst)
                    vstt(dst, X[:, :, 11 + 2 * ch], cw[1, 0], dst)
                    vstt(dst, X[:, :, 12 + 2 * ch], cw[1, 1], dst)
                    vstt(dst, X[:, :, 13 + 2 * ch], cw[1, 2], dst)

                # --- relu(U) and relu(C1) in one DVE 2x-mode op ---
                vrelu(UC[:])

                # --- conv2: S = c20*C1[t] + c21*C1[t+1] + c2b ---
                aff(S[:], C1[:, 0:2 * c], c2[0], c2b)
                vstt(S[:], C1[:, c:3 * c], c2[1], S[:])

                # --- T = relu(S) + U  (U already relu'd; fused on DVE) ---
                vstt(S[:], S[:], 0.0, U[:], op0=MAX, op1=ADD)

                # --- fc_emb ---
                aff(E[:, 0:c], S[:, 0:c], we[0, 0], be[0])
                aff(E[:, c:2 * c], S[:, 0:c], we[1, 0], be[1])
                vstt(E[:, 0:c], S[:, c:2 * c], we[0, 1], E[:, 0:c])
                vstt(E[:, c:2 * c], S[:, c:2 * c], we[1, 1], E[:, c:2 * c])
                vrelu(E[:])

                # --- deconv1 -> D ---
                aff(D[:, 0:c], E[:, 0:c], dv[0], d1b, func=RELU)
                aff(D[:, 2 * c:3 * c], E[:, c:2 * c], dv[1], d1b, func=RELU)
                aff(D[:, c:2 * c], E[:, 0:c], dv[1], d1b)
                vstt(D[:, c:2 * c], E[:, c:2 * c], dv[0], D[:, c:2 * c])
                aff(D[:, c:2 * c], D[:, c:2 * c], 1.0, 0.0, func=RELU)

                # --- deconv2 -> Y row-major (c, 7) ---
                aff(Yv[:, 0, :], D[:, 0:c], dw[0], d2b)
                aff(Yv[:, 1, :], D[:, 0:c], dw[1], d2b)
                aff(Yv[:, 3, :], D[:, c:2 * c], dw[1], d2b)
                aff(Yv[:, 5, :], D[:, 2 * c:3 * c], dw[1], d2b)
                aff(Yv[:, 6, :], D[:, 2 * c:3 * c], dw[2], d2b)
                aff(Yv[:, 2, :], D[:, 0:c], dw[2], d2b)
                vstt(Yv[:, 2, :], D[:, c:2 * c], dw[0], Yv[:, 2, :])
                aff(Yv[:, 4, :], D[:, c:2 * c], dw[2], d2b)
                vstt(Yv[:, 4, :], D[:, 2 * c:3 * c], dw[0], Yv[:, 4, :])

                nc.sync.dma_start(out=yd, in_=Y[:])

    _split_multi_waits(nc)
    return nc


def _split_multi_waits(nc):
    """Walrus codegen accepts at most ONE sync-wait per instruction; hoist
    extra waits onto standalone same-engine NoOps placed just before."""
    import concourse.mybir as mybir

    n = 0
    for fn in nc.m.functions:
        for bb in fn.blocks:
            out = []
            for ins in bb.instructions:
                si = getattr(ins, "sync_info", None)
                waits = list(si.on_wait) if si and si.on_wait else []
                if len(waits) > 1:
                    for w in waits[:-1]:
                        nop = mybir.InstNoOp(name=f"waitnop-{n}", ins=[], outs=[])
                        n += 1
                        nop.engine = ins.engine
                        nop.sync_info = mybir.SyncInfo(on_wait=[w], on_update=[])
                        out.append(nop)
                    ins.sync_info = mybir.SyncInfo(
                        on_wait=[waits[-1]], on_update=list(si.on_update or [])
                    )
                out.append(ins)
            bb.instructions = out


LAST_RESULTS = None  # test harness introspection (exec_time_ns, profile)


def _run(nc, in_maps, core_ids, trace=False):
    global LAST_RESULTS
    from concourse.bass_utils import run_bass_kernel_spmd

    LAST_RESULTS = run_bass_kernel_spmd(nc, in_maps, core_ids, trace=trace)
    return LAST_RESULTS


def kernel(**inputs) -> np.ndarray:
    x = np.asarray(inputs["x"], dtype=np.float32)
    weights = {
        k: np.asarray(v, dtype=np.float32) for k, v in inputs.items() if k != "x"
    }
    assert x.shape == (B, 18), x.shape

    nc = _build(weights)

    xp = np.zeros((PADDED, 18), dtype=np.float32)
    xp[:B] = x
    shards = xp.reshape(N_CORES, ROWS_PER_CORE, 18)
    in_maps = [{"x": np.ascontiguousarray(shards[i])} for i in range(N_CORES)]

    res = _run(nc, in_maps, list(range(N_CORES)))
    outs = [np.asarray(res.results[i]["y"]) for i in range(N_CORES)]
    y = np.concatenate(outs, axis=0)[:B]
    return np.ascontiguousarray(y.reshape(B, 1, 7))
